# revision 1
# baseline (speedup 1.0000x reference)
"""Trainium2 Bass kernel for CustomEncoderWithAction (gnn_message_passing).

Strategy (8 NeuronCores, full inputs in / full output out):
  * Key insight: the reference computes the [N,N] pairwise pooling for all
    N=1536 agents, but only pooled[robot_idx] (B=192 rows) is consumed.
    We compute pooled ONLY for the 192 robot agents, sharded 24 per core.
  * Pooling layer-1 decomposes: u1[i,j,:] = a[i,:] + b[j,:] with
      a_i = pos_i @ Wc,  b_j = -pos_j @ Wc + h_j @ W_p1b + (b_sp@W_p1a + b_p1),
      Wc = W_sp @ W_p1[:EMB]   (weight folding done host-side).
    relu(a_i + b_j) is one tensor_scalar op per pair of agents (GPSIMD, bf16).
  * Layer-2 (64->16) runs on the TensorEngine with a block-diagonal
    [128, 32] weight (2 agents per pass), 8 agents' z stacked in PSUM [128, N].
  * Neighbor mask enters as an additive -2^30 * (1-m) via a K=8 matmul that
    STARTS each PSUM bank (overlaps the LSTM); pair matmuls accumulate on top;
    pooled = relu(max_j z + b_p2) since relu commutes with max and
    no-neighbor rows come out 0 automatically.
  * LSTM encoder (T=8, all N agents) is replicated on every core, N-major:
    lhsT = [hT; traj_rel_t; 1] tiles (x-embedding folded: W_he@W_ih),
    sigmoid/tanh on ACT, state update on DVE, h transposed back to
    feature-major via PE transposes; elementwise split in halves to pipeline.
  * Fusion MLP runs on the core's own 24 robots; host concatenates.
"""

import numpy as np
import ml_dtypes
from contextlib import ExitStack

import concourse.bass as bass
import concourse.bacc as bacc
import concourse.tile as tile
from concourse import mybir
from concourse.bass_utils import run_bass_kernel_spmd

F32 = mybir.dt.float32
BF16 = mybir.dt.bfloat16
AL = mybir.AluOpType
AF = mybir.ActivationFunctionType
AX = mybir.AxisListType

T, N, B, A_DIM, H, EMB, MID, F = 8, 1536, 192, 2, 16, 16, 64, 256
NC_ = 8          # cores
BPC = B // NC_   # 24 robots per core
NPAIR = BPC // 2  # 12
NGRP = BPC // 8   # 3
BIG = float(2 ** 30)
CH = 512          # psum free chunk
NCH = N // CH     # 3

bf16 = ml_dtypes.bfloat16


def _din(nc, name, shape, dt):
    return nc.dram_tensor(name, list(shape), dt, kind="ExternalInput").ap()


_IN_SPECS = [
    # name, shape, dtype
    ("trajrelT", [3, T, N], BF16),
    ("W_cat2", [19, 4 * H], BF16),
    ("posT", [2, N], BF16),
    ("BD", [128, 32], BF16),
    ("msel", [8, 128], BF16),
    ("ident", [128, 128], BF16),
    ("WcP", [2, MID], BF16),
    ("WcN", [2, MID], BF16),
    ("cvecT", [1, MID], BF16),
    ("W_p1b", [H, MID], BF16),
    ("W_emb", [4, H], BF16),
    ("b_embT", [H, 1], F32),
    ("b_p2T", [H, 1], F32),
    ("W_fca", [48, F], BF16),
    ("sel", [128, 8, H], F32),
    ("nm", [8, NGRP, N], BF16),
    ("pIcE", [2, NPAIR], BF16),
    ("pIcO", [2, NPAIR], BF16),
    ("r_goalT", [2, BPC], F32),
    ("posrobT", [2, BPC], F32),
    ("actionT", [2, BPC], BF16),
    ("rpo", [16, BPC], BF16),
]


def _build():
    nc = bacc.Bacc("TRN2", target_bir_lowering=False, debug=False)
    a = {nm: _din(nc, nm, sh, dt) for nm, sh, dt in _IN_SPECS}
    a["out"] = nc.dram_tensor("out", [BPC, F], F32, kind="ExternalOutput").ap()
    with tile.TileContext(nc) as tc, ExitStack() as ctx:
        _emit(ctx, tc, nc, a)
    nc.compile()
    return nc


def _emit(ctx, tc, nc, a):
    sb = ctx.enter_context(tc.tile_pool(name="sb", bufs=1))
    qs = [nc.sync, nc.scalar, nc.gpsimd]
    qi = [0]

    def dma(out, in_):
        eng = qs[qi[0] % len(qs)]
        qi[0] += 1
        eng.dma_start(out=out, in_=in_)

    # lhsT for LSTM steps: rows 0-15 hT, 16-17 traj_rel_t, 18 ones
    xh = sb.tile([19, T, N], BF16, name="xh")
    nc.sync.dma_start(out=xh[16:19, :, :], in_=a["trajrelT"])
    W_cat2 = sb.tile([19, 4 * H], BF16, name="W_cat2")
    nc.scalar.dma_start(out=W_cat2, in_=a["W_cat2"])
    nc.vector.memset(xh[0:16, 0, :], 0.0)  # h0 = 0

    def load(name, shape, dt):
        t = sb.tile(list(shape), dt, name=f"sb_{name}")
        dma(out=t, in_=a[name])
        return t

    ident_sb = load("ident", [128, 128], BF16)
    posT_sb = load("posT", [2, N], BF16)
    BD_sb = load("BD", [128, 32], BF16)
    msel_sb = load("msel", [8, 128], BF16)
    WcP_sb = load("WcP", [2, MID], BF16)
    WcN_sb = load("WcN", [2, MID], BF16)
    cvecT_sb = load("cvecT", [1, MID], BF16)
    W_p1b_sb = load("W_p1b", [H, MID], BF16)
    W_emb_sb = load("W_emb", [4, H], BF16)
    b_embT_sb = load("b_embT", [H, 1], F32)
    W_fca_sb = load("W_fca", [48, F], BF16)
    sel_sb = load("sel", [128, 8, H], F32)
    nm_sb = load("nm", [8, NGRP, N], BF16)
    pIcE_sb = load("pIcE", [2, NPAIR], BF16)
    pIcO_sb = load("pIcO", [2, NPAIR], BF16)
    rg_sb = load("r_goalT", [2, BPC], F32)
    pr_sb = load("posrobT", [2, BPC], F32)

    ones_row = sb.tile([1, N], BF16, name="ones_row")
    nc.vector.memset(ones_row, 1.0)
    # prefetch the sigmoid/tanh ACT table set while input DMAs run
    warm = sb.tile([1, 2], F32, name="warm")
    nc.vector.memset(warm, 0.0)
    nc.scalar.activation(out=warm, in_=warm, func=AF.Sigmoid)

    # ---------- LSTM over T steps (replicated, all N agents) ----------
    ench = sb.tile([H, N], BF16, name="ench")      # final hidden, feature-major
    c_sb = sb.tile([128, 12, H], F32, name="c_sb")
    nc.vector.memset(c_sb, 0.0)
    sg = sb.tile([128, 12, 48], BF16, name="sg")
    tg = sb.tile([128, 12, H], BF16, name="tg")
    th = sb.tile([128, 12, H], BF16, name="th")
    hn = sb.tile([128, 12, H], BF16, name="hn")
    t1 = sb.tile([128, 12, H], BF16, name="t1")
    t2 = sb.tile([128, 12, H], F32, name="t2")

    import os
    HK = int(os.environ.get("LSTM_HK", "6"))  # tiles per chunk

    with tc.tile_pool(name="lstm_g", bufs=2, space="PSUM") as gpool, \
         tc.tile_pool(name="lstm_tp", bufs=4, space="PSUM") as tpool:
        for t in range(T):
            g_ps = gpool.tile([128, 12, 4 * H], F32, name="g_ps")
            dest = xh[0:16, t + 1, :] if t < T - 1 else ench
            for half in range(12 // HK):
                hs = slice(HK * half, HK * (half + 1))
                for k in range(HK * half, HK * (half + 1)):
                    nc.tensor.matmul(
                        g_ps[:, k, :], xh[:, t, 128 * k:128 * (k + 1)], W_cat2,
                        start=True, stop=True)
                # gate cols: [i(0:16), f(16:32), o(32:48), g(48:64)]
                nc.scalar.activation(
                    out=sg[:, hs, :], in_=g_ps[:, hs, 0:48], func=AF.Sigmoid)
                nc.scalar.activation(
                    out=tg[:, hs, :], in_=g_ps[:, hs, 48:64], func=AF.Tanh)
                nc.vector.tensor_tensor(
                    out=t1[:, hs, :], in0=sg[:, hs, 0:16], in1=tg[:, hs, :],
                    op=AL.mult)
                nc.vector.tensor_tensor(
                    out=t2[:, hs, :], in0=sg[:, hs, 16:32], in1=c_sb[:, hs, :],
                    op=AL.mult)
                nc.vector.tensor_tensor(
                    out=c_sb[:, hs, :], in0=t1[:, hs, :], in1=t2[:, hs, :],
                    op=AL.add)
                nc.scalar.activation(
                    out=th[:, hs, :], in_=c_sb[:, hs, :], func=AF.Tanh)
                nc.vector.tensor_tensor(
                    out=hn[:, hs, :], in0=sg[:, hs, 32:48], in1=th[:, hs, :],
                    op=AL.mult)
                tp = tpool.tile([H, HK * 128], BF16, name="tp")
                for q in range(HK):
                    nc.tensor.transpose(
                        tp[:, 128 * q:128 * (q + 1)],
                        hn[:, HK * half + q, :], ident_sb)
                half_off = HK * 128 * half
                npiece = max(1, (HK * 128) // 384)
                pw = HK * 128 // npiece
                for piece in range(npiece):
                    po = pw * piece
                    nc.vector.tensor_copy(
                        out=dest[:, half_off + po:half_off + po + pw],
                        in_=tp[:, po:po + pw])

    # ---------- bT2 [128, N]: b_j stacked twice on partitions ----------
    bT2 = sb.tile([128, N], BF16, name="bT2")
    with tc.tile_pool(name="b_ps", bufs=2, space="PSUM") as bpool:
        for ch in range(NCH):
            s = slice(CH * ch, CH * (ch + 1))
            b_ps = bpool.tile([128, CH], F32, name="b_ps")
            for hb in (0, 64):
                nc.tensor.matmul(b_ps[hb:hb + 64, :], WcN_sb, posT_sb[:, s],
                                 start=True, stop=False,
                                 tile_position=(0, hb), skip_group_check=True)
                nc.tensor.matmul(b_ps[hb:hb + 64, :], W_p1b_sb, ench[:, s],
                                 start=False, stop=False,
                                 tile_position=(0, hb), skip_group_check=True)
                nc.tensor.matmul(b_ps[hb:hb + 64, :], cvecT_sb, ones_row[:, s],
                                 start=False, stop=True,
                                 tile_position=(0, hb), skip_group_check=True)
            nc.vector.tensor_copy(out=bT2[:, s], in_=b_ps)

        # aT2 [128, NPAIR]: column p = [a(robot 2p); a(robot 2p+1)]
        a_ps = bpool.tile([128, NPAIR], F32, name="a_ps")
        nc.tensor.matmul(a_ps[0:64, :], WcP_sb, pIcE_sb, start=True, stop=True)
        nc.tensor.matmul(a_ps[64:128, :], WcP_sb, pIcO_sb, start=True, stop=True)
        aT2 = sb.tile([128, NPAIR], F32, name="aT2")
        nc.vector.tensor_copy(out=aT2, in_=a_ps)

    # ---------- pairwise pooling ----------
    pool_parts = sb.tile([128, NGRP, NCH], F32, name="pool_parts")
    pool_pre = sb.tile([128, NGRP], F32, name="pool_pre")
    with tc.tile_pool(name="ru_pool", bufs=4) as rupool, \
         tc.tile_pool(name="z_ps", bufs=2, space="PSUM") as zpool:
        for g in range(NGRP):
            z = zpool.tile([128, N], F32, name="z", tag="z")
            for ch in range(NCH):
                s = slice(CH * ch, CH * (ch + 1))
                nc.tensor.matmul(
                    z[:, s], msel_sb, nm_sb[:, g, s], start=True, stop=False,
                    skip_group_check=True)
            for ai in range(4):
                p = 4 * g + ai
                ru = rupool.tile([128, N], BF16, name="ru", tag="ru")
                if ai % 2 == 0:
                    nc.vector.tensor_scalar(
                        out=ru, in0=bT2, scalar1=aT2[:, p:p + 1], scalar2=0.0,
                        op0=AL.add, op1=AL.max)
                else:
                    nc.scalar.activation(
                        out=ru, in_=bT2, func=AF.Relu,
                        bias=aT2[:, p:p + 1])
                for ch in range(NCH):
                    s = slice(CH * ch, CH * (ch + 1))
                    nc.tensor.matmul(
                        z[32 * ai:32 * (ai + 1), s], BD_sb, ru[:, s],
                        start=False, stop=(ai == 3),
                        tile_position=(0, 32 * ai), skip_group_check=True)
            for ch in range(NCH):
                s = slice(CH * ch, CH * (ch + 1))
                nc.vector.tensor_reduce(
                    out=pool_parts[:, g, ch:ch + 1], in_=z[:, s],
                    axis=AX.X, op=AL.max)
    nc.vector.tensor_reduce(
        out=pool_pre, in_=pool_parts, axis=AX.X, op=AL.max)

    # ---------- fusion MLP for this core's 24 robots ----------
    fuseT = sb.tile([48, BPC], BF16, name="fuseT")
    dma(out=fuseT[16:32, :], in_=a["rpo"])  # r_pose, ones row, zeros
    bp2_48 = sb.tile([48, 1], F32, name="bp2_48")
    dma(out=bp2_48[32:48, :], in_=a["b_p2T"])
    spT = sb.tile([4, BPC], BF16, name="spT")
    dma(out=spT[2:4, :], in_=a["actionT"])
    nc.vector.tensor_tensor(out=spT[0:2, :], in0=rg_sb, in1=pr_sb, op=AL.subtract)
    out_sb = sb.tile([BPC, F], F32, name="out_sb")
    with tc.tile_pool(name="f_ps", bufs=1, space="PSUM") as fpool:
        se_ps = fpool.tile([H, BPC], F32, name="se_ps")
        nc.tensor.matmul(se_ps, W_emb_sb, spT, start=True, stop=True)
        nc.scalar.activation(
            out=fuseT[0:16, :], in_=se_ps, func=AF.Relu, bias=b_embT_sb)
        pg_ps = fpool.tile([48, BPC], F32, name="pg_ps")
        pg_v = pg_ps.rearrange("p (c l) -> p c l", l=8)
        for l in range(8):
            nc.tensor.matmul(
                pg_v[32:48, :, l], sel_sb[:, l, :], pool_pre,
                start=True, stop=True)
        nc.scalar.activation(
            out=fuseT[32:48, :], in_=pg_ps[32:48, :], func=AF.Relu,
            bias=bp2_48[32:48, :])
        o_ps = fpool.tile([BPC, F], F32, name="o_ps")
        nc.tensor.matmul(o_ps, fuseT, W_fca_sb, start=True, stop=True)
        nc.scalar.activation(out=out_sb, in_=o_ps, func=AF.Relu)
    nc.sync.dma_start(out=a["out"], in_=out_sb)


# ------------------------------------------------------------------
# host side
# ------------------------------------------------------------------
_NC_CACHE = None


def _gates_reorder(w):
    # torch gate order i,f,g,o (16 each) -> i,f,o,g
    i, f, g, o = np.split(np.asarray(w, np.float32), 4, axis=-1)
    return np.concatenate([i, f, o, g], axis=-1)


def _bf(x):
    return np.ascontiguousarray(np.asarray(x, np.float32).astype(bf16))


def _f32(x):
    return np.ascontiguousarray(np.asarray(x, np.float32))


def kernel(obs_traj_pos, traj_rel, neigh_index, robot_idx, r_goal, r_pose,
           action, W_he, b_he, W_ih, W_hh, b_ih, b_hh, W_sp, b_sp, W_p1, b_p1,
           W_p2, b_p2, W_emb, b_emb, W_fc, b_fc):
    global _NC_CACHE
    obs_traj_pos = np.asarray(obs_traj_pos, np.float32)
    traj_rel = np.asarray(traj_rel, np.float32)
    neigh_index = np.asarray(neigh_index)
    robot_idx = np.asarray(robot_idx)
    pos = obs_traj_pos[-1]                        # [N, 2]
    f = _f32

    # fold x-embedding into the recurrent matmul:
    #   gates = traj_rel@(W_he W_ih) + h@W_hh + (b_ih + b_he@W_ih + b_hh)
    W_heih = f(W_he) @ f(W_ih)
    bias = f(b_ih) + f(b_he) @ f(W_ih) + f(b_hh)
    W_cat2 = np.zeros((19, 64), np.float32)
    W_cat2[0:16] = _gates_reorder(W_hh)
    W_cat2[16:18] = _gates_reorder(W_heih)
    W_cat2[18] = _gates_reorder(bias)

    Wc = f(W_sp) @ f(W_p1)[:EMB]                  # [2, 64]
    cvec = f(b_sp) @ f(W_p1)[:EMB] + f(b_p1)      # [64]

    rep = dict(
        trajrelT=_bf(np.concatenate(
            [np.transpose(traj_rel, (2, 0, 1)),
             np.ones((1, T, N), np.float32)], axis=0)),
        W_cat2=_bf(W_cat2),
        posT=_bf(pos.T),
        WcP=_bf(Wc),
        WcN=_bf(-Wc),
        cvecT=_bf(cvec[None, :]),
        W_p1b=_bf(f(W_p1)[EMB:]),
        W_emb=_bf(W_emb),
        b_embT=f(b_emb)[:, None].copy(),
        b_p2T=f(b_p2)[:, None].copy(),
        ident=_bf(np.eye(128)),
        sel=_f32(np.eye(128).reshape(128, 8, H)),
    )
    bd = np.zeros((128, 32), np.float32)
    bd[0:64, 0:16] = W_p2
    bd[64:128, 16:32] = W_p2
    rep["BD"] = _bf(bd)
    ms = np.zeros((8, 128), np.float32)
    for l in range(8):
        ms[l, 16 * l:16 * (l + 1)] = -BIG
    rep["msel"] = _bf(ms)
    wf = np.zeros((48, F), np.float32)
    wf[0:16] = W_fc[0:16]        # spatial_emb rows
    wf[16:21] = W_fc[32:37]      # r_pose rows
    wf[21] = b_fc                # bias row (matched by ones in rpo row 5->21)
    wf[32:48] = W_fc[16:32]      # pooled rows
    rep["W_fca"] = _bf(wf)

    in_maps = []
    for c in range(NC_):
        I = robot_idx[BPC * c:BPC * (c + 1)]
        nm = np.zeros((8, NGRP, N), np.float32)
        for g in range(NGRP):
            for l in range(8):
                nm[l, g] = 1.0 - (neigh_index[I[8 * g + l]] > 0)
        m = dict(rep)
        m["nm"] = _bf(nm)
        m["pIcE"] = _bf(pos[I[0::2]].T)
        m["pIcO"] = _bf(pos[I[1::2]].T)
        m["r_goalT"] = f(r_goal)[BPC * c:BPC * (c + 1)].T.copy()
        m["posrobT"] = _f32(pos[I].T)
        m["actionT"] = _bf(f(action)[BPC * c:BPC * (c + 1)].T)
        rpo = np.zeros((16, BPC), np.float32)
        rpo[0:5] = f(r_pose)[BPC * c:BPC * (c + 1)].T
        rpo[5] = 1.0
        m["rpo"] = _bf(rpo)
        in_maps.append(m)

    if _NC_CACHE is None:
        _NC_CACHE = _build()
    res = run_bass_kernel_spmd(_NC_CACHE, in_maps, core_ids=list(range(NC_)))
    out = np.concatenate([r["out"] for r in res.results], axis=0)
    return out.astype(np.float32)


if __name__ == "__main__":
    import reference
    inp = {k: np.asarray(v) for k, v in reference.setup_inputs().items()}
    got = kernel(**inp)
    exp = np.asarray(reference.reference(**inp))
    err = np.abs(got - exp)
    print("max abs err", err.max(), "scale", np.abs(exp).max())
    print("rel-of-max", err.max() / np.abs(exp).max())

